# revision 26
# baseline (speedup 1.0000x reference)
"""Trainium2 Bass kernel for LocalXLAttention (chunk-summed variant).

Math: the reference einsum sums over the chunk index z, so every query
attends to the same three [w, dh] K/V matrices built from chunk sums:
  K_prev = S_k - k_chunk[C-1], K_cur = S_k, K_next = S_k - k_chunk[0]
(and identically for V), where S_k = sum_c k_chunk[c].  The computation
collapses to, per sequence position l and head h:
  attn[l,h,:]  = qp[l,h,:] @ KbigT          (KbigT: [dh, 3w])
  probs        = softmax(attn, axis=-1)
  ctx[l,h,:]   = probs[l,h,:] @ Vbig        (Vbig:  [3w, dh])
  out          = ctx.reshape(L, dm) @ Wc
with the scores factored per block:
  exp(u - a) = exp(u)*exp(-a),  u = qp.S^T,  a = qp.c7^T,  b = qp.c0^T
so the a/b exponentials of the first head pairs run while the kv chunk-sum
S is still streaming in (the exp unit is the overall bottleneck at ~12.6M
exps/core; it must start early and never stall).

Sharding: L=4096 is split 512 rows per core across 8 NeuronCores
(data-parallel over the sequence; no collectives).  Each core redundantly
computes the tiny chunk-summed K/V from the full kv input.  All inputs are
cast to bf16 on the host; chunks 0 and 7 of kv are fetched first as
dedicated tiles (they gate the a/b scores), the middle 6 chunks stream
behind them.

Layouts are fully transposed ([j, l] / [he, l]) so no on-device
activation transposes are needed; softmax normalization is deferred to
the context via an all-ones column of Vbig (row 64 of the ctx psum
accumulates the denominator for free).

PSUM budget (8 banks):
  tag "qk"  3 x [128,1024] f32  (6 banks) - QK/exp ping-pong-pang, QP
                                            projection, out-proj partials
  tag "ctx" 2 x [128, 512] f32  (2 banks) - per-pair ctx accumulators,
                                            kv projections, vbig transposes

Pipeline per head pair: QK (TensorE, row-tiled pairs) -> exp (ScalarE)
-> PV (TensorE, psum-accumulated).  Pair ctx is raw-evicted to SBUF at
pair end (fast psum release), normalized lazily (approx reciprocal +
gpsimd broadcast), and the out-projection is spread across the next
pair's j-loop; partial outputs accumulate in SBUF (bf16) via VectorE.
"""

import sys
for _p in ('/opt/pypackages', '/opt/trn_rl_repo'):
    if _p not in sys.path:
        sys.path.insert(0, _p)

import numpy as np
import ml_dtypes

import concourse.bass as bass
import concourse.bacc as bacc
import concourse.tile as tile
from concourse import mybir
from concourse.bass_utils import run_bass_kernel_spmd
from concourse.masks import make_identity

F32 = mybir.dt.float32
BF16 = mybir.dt.bfloat16
AF = mybir.ActivationFunctionType

N_CORES = 8
L = 4096          # full sequence
LS = L // N_CORES # 512 rows per core
DM = 1024
NH = 16
DH = 64
W = 512           # chunk width
C = L // W        # 8 chunks
J3 = 3 * W        # 1536 softmax width
NJ = J3 // 128    # 12 j-chunks
DMT = DM // 128   # 8 dm-chunks
MID = L - 2 * W   # 3072 middle columns (chunks 1..6)
N_EAB = 2         # head pairs that run the decomposed (early) schedule


def build_nc():
    nc = bacc.Bacc(None, target_bir_lowering=False)

    # all operands arrive host-packed as [128, DMT*cols]:
    # packed[p, d*cols + c] = orig[128*d + p, c] -- every DMA is a fully
    # contiguous [128, N] transfer (strided dram APs ran ~3x slower).
    qtp = nc.dram_tensor("qtp", [128, DMT * LS], BF16, kind="ExternalInput")
    kvm = nc.dram_tensor("kvm", [DM, MID], BF16, kind="ExternalInput")
    kvc0 = nc.dram_tensor("kvc0", [128, DMT * W], BF16, kind="ExternalInput")
    kvc7 = nc.dram_tensor("kvc7", [128, DMT * W], BF16, kind="ExternalInput")
    wqlo = nc.dram_tensor("wqlo", [128, DMT * 512], BF16, kind="ExternalInput")
    wqhi = nc.dram_tensor("wqhi", [128, DMT * 512], BF16, kind="ExternalInput")
    wkvp = nc.dram_tensor("wkvp", [128, DMT * 2 * DH], BF16, kind="ExternalInput")
    wcp = nc.dram_tensor("wcp", [128, DMT * DM], BF16, kind="ExternalInput")
    # bf16 output (cast during the SWDGE DMA) halves the output-write
    # tail; the host upcasts to fp32.
    out = nc.dram_tensor("out", [LS, DM], BF16, kind="ExternalOutput")

    with tile.TileContext(nc) as tc:
        with tc.tile_pool(name="weights", bufs=1) as wpool, \
             tc.tile_pool(name="qt", bufs=1) as qpool, \
             tc.tile_pool(name="stream", bufs=7) as stpool, \
             tc.tile_pool(name="kvc", bufs=2) as kvcpool, \
             tc.tile_pool(name="ksum", bufs=8) as kspool, \
             tc.tile_pool(name="qpt", bufs=4) as qptpool, \
             tc.tile_pool(name="small", bufs=1) as spool, \
             tc.tile_pool(name="probs", bufs=8) as ppool, \
             tc.tile_pool(name="eab", bufs=16) as eabpool, \
             tc.tile_pool(name="craw", bufs=2) as crpool, \
             tc.tile_pool(name="ctxu", bufs=2) as cupool, \
             tc.tile_pool(name="outacc", bufs=4) as opool, \
             tc.tile_pool(name="misc", bufs=2) as mpool, \
             tc.tile_pool(name="ps", bufs=1, space="PSUM") as pspool:

            # ---------- phase 0: input DMAs (both HWDGE rings) ----------
            # ring FIFO order is the priority order:
            #   sync:   qt, Wq-lo, kvc0, Wq-hi, stm0, stm2, Wc
            #   scalar: wkv, kvc7, stm1, stm3..stm7
            qt3 = qpool.tile([128, DMT * LS], BF16, tag="qt")
            nc.sync.dma_start(out=qt3, in_=qtp[:, :])
            wkv3 = wpool.tile([128, DMT * 2 * DH], BF16, tag="wkv")
            nc.scalar.dma_start(out=wkv3, in_=wkvp[:, :])
            wqlo3 = wpool.tile([128, DMT * 512], BF16, tag="wqlo")
            nc.sync.dma_start(out=wqlo3, in_=wqlo[:, :])
            kvc73 = kvcpool.tile([128, DMT * W], BF16, tag="kvc")
            nc.scalar.dma_start(out=kvc73, in_=kvc7[:, :])
            kvc03 = kvcpool.tile([128, DMT * W], BF16, tag="kvc")
            nc.sync.dma_start(out=kvc03, in_=kvc0[:, :])
            wqhi3 = wpool.tile([128, DMT * 512], BF16, tag="wqhi")
            nc.sync.dma_start(out=wqhi3, in_=wqhi[:, :])
            stm_sb = []
            for d in range(DMT):
                t = stpool.tile([128, MID], BF16, tag="st", name=f"stm{d}")
                eng = nc.sync if d in (0, 2) else nc.scalar
                eng.dma_start(out=t, in_=kvm[128 * d:128 * (d + 1), :])
                stm_sb.append(t)
            wc3 = wpool.tile([128, DMT * DM], BF16, tag="wc")
            nc.sync.dma_start(out=wc3, in_=wcp[:, :])
            qt_sb = [qt3[:, LS * d:LS * (d + 1)] for d in range(DMT)]
            wkv_sb = [wkv3[:, 2 * DH * d:2 * DH * (d + 1)] for d in range(DMT)]
            kvc7_sb = [kvc73[:, W * d:W * (d + 1)] for d in range(DMT)]
            kvc0_sb = [kvc03[:, W * d:W * (d + 1)] for d in range(DMT)]
            wc_sb = [wc3[:, DM * d:DM * (d + 1)] for d in range(DMT)]

            ident = spool.tile([128, 128], BF16, tag="ident")
            make_identity(nc, ident)

            # preload the exp table so the first real exp isn't delayed ~2.7us
            dummy = mpool.tile([1, 8], F32, tag="dummy")
            nc.scalar.activation(dummy, ident[0:1, 0:8], AF.Exp, scale=1.0)

            # ---------- chunk 7 / chunk 0 projections (early) ----------
            # [128, 512] psum: k rows 0:64, v rows 64:128 (v matmuls are
            # col-tiled to base partition 64), accumulated over dm-chunks.
            kv7p = pspool.tile([128, W], F32, tag="ctx", bufs=1, name="kv7p")
            kv0p = pspool.tile([128, W], F32, tag="op", bufs=1, name="kv0p")
            def kv_proj(srcs, dst):
                for d in range(DMT):
                    nc.tensor.matmul(dst[0:DH, :], wkv_sb[d][:, 0:DH],
                                     srcs[d], start=(d == 0), stop=(d == DMT - 1))
                    nc.tensor.matmul(dst[DH:128, :], wkv_sb[d][:, DH:2 * DH],
                                     srcs[d], start=(d == 0), stop=(d == DMT - 1))

            kv_proj(kvc7_sb, kv7p)

            # a/b-score lhsT tiles (rows duplicated for the row-tiled pair)
            # + v7/v0 for the Vbig transposes
            k7b = spool.tile([128, W], BF16, tag="k7b")
            k0b = spool.tile([128, W], BF16, tag="k0b")
            v7_sb = spool.tile([DH, W], BF16, tag="v7")
            v0_sb = spool.tile([DH, W], BF16, tag="v0")
            nc.vector.tensor_copy(k7b[0:DH, :], kv7p[0:DH, :])
            nc.vector.tensor_copy(k7b[DH:128, :], k7b[0:DH, :])
            nc.vector.tensor_copy(v7_sb, kv7p[DH:128, :])

            # ---------- QP projection (by head quads) ----------
            qpt_sb = []

            def qp_quad(t4):
                ps = pspool.tile([128, 1024], F32, tag="op", bufs=1,
                                 name=f"qp{t4}")
                wqt = wqlo3 if t4 < 2 else wqhi3
                for half in range(2):
                    hd = (2 * t4 + half) % 4
                    for d in range(DMT):
                        nc.tensor.matmul(
                            ps[:, 512 * half:512 * (half + 1)],
                            wqt[:, 512 * d + 128 * hd:512 * d + 128 * (hd + 1)],
                            qt_sb[d],
                            start=(d == 0), stop=(d == DMT - 1))
                sb = qptpool.tile([128, 1024], BF16, tag="qpt", name=f"qpt{t4}")
                nc.vector.tensor_copy(sb, ps)
                qpt_sb.append(sb)

            def qk_mm_pair(lhsT, qpt, csl, name):
                qk = pspool.tile([128, 1024], F32, tag="qk", bufs=2, name=name)
                nc.tensor.matmul(qk[:, 0:W], lhsT[0:DH, :],
                                 qpt[0:DH, csl], start=True, stop=True)
                nc.tensor.matmul(qk[:, W:2 * W], lhsT[DH:128, :],
                                 qpt[DH:2 * DH, csl], start=True, stop=True)
                return qk

            # ---------- alpha phase: a/b exponentials of pairs 0..N_EAB-1 --
            # Ea = exp(-0.125*a), Eb = exp(-0.125*b); multiplied by
            # Eu = exp(0.125*u) later, once the chunk-sum S lands.
            ea_t = [[None] * 4 for _ in range(N_EAB)]
            eb_t = [[None] * 4 for _ in range(N_EAB)]

            def alpha_block(p, blk):
                qpt = qpt_sb[p // 2]
                csl = slice(512 * (p % 2), 512 * (p % 2) + W)
                lhsT, store = ((k7b, ea_t) if blk == 0 else (k0b, eb_t))
                for jj in range(4):
                    qk = qk_mm_pair(lhsT[:, 128 * jj:128 * (jj + 1)], qpt, csl,
                                    f"abqk{p}_{blk}_{jj}")
                    e = eabpool.tile([128, 1024], BF16, tag="eab",
                                     name=f"e{p}_{blk}_{jj}")
                    nc.scalar.activation(e, qk, AF.Exp, scale=-0.125)
                    store[p][jj] = e

            qp_quad(0)
            alpha_block(0, 0)
            kv_proj(kvc0_sb, kv0p)
            nc.vector.tensor_copy(k0b[0:DH, :], kv0p[0:DH, :])
            nc.vector.tensor_copy(k0b[DH:128, :], k0b[0:DH, :])
            nc.vector.tensor_copy(v0_sb, kv0p[DH:128, :])
            alpha_block(0, 1)
            alpha_block(1, 0)
            alpha_block(1, 1)

            # ---------- chunk-sum tree (middle chunks + c0 + c7) ----------
            ks_sb = []
            for d in range(DMT):
                stm = stm_sb[d]
                nc.vector.tensor_add(stm[:, 0:1536], stm[:, 0:1536],
                                     stm[:, 1536:3072])
                nc.vector.tensor_add(stm[:, 0:512], stm[:, 0:512],
                                     stm[:, 512:1024])
                ks = kspool.tile([128, W], BF16, tag="ks", name=f"ks{d}")
                nc.vector.tensor_add(ks, stm[:, 0:512], stm[:, 1024:1536])
                nc.vector.tensor_add(ks, ks, kvc0_sb[d])
                nc.vector.tensor_add(ks, ks, kvc7_sb[d])
                ks_sb.append(ks)

            ksump = pspool.tile([128, W], F32, tag="op", bufs=1, name="ksump")
            for d in range(DMT):
                nc.tensor.matmul(ksump[0:DH, :], wkv_sb[d][:, 0:DH],
                                 ks_sb[d], start=(d == 0), stop=(d == DMT - 1))
                nc.tensor.matmul(ksump[DH:128, :], wkv_sb[d][:, DH:2 * DH],
                                 ks_sb[d], start=(d == 0), stop=(d == DMT - 1))

            # ---------- KbigT [128, 1536] = [prev | cur | next] ----------
            kbig = spool.tile([128, J3], BF16, tag="kbig")
            nc.vector.tensor_sub(kbig[0:DH, 0:W], ksump[0:DH, :], k7b[0:DH, :])
            nc.vector.tensor_copy(kbig[0:DH, W:2 * W], ksump[0:DH, :])
            nc.vector.tensor_sub(kbig[0:DH, 2 * W:3 * W], ksump[0:DH, :],
                                 k0b[0:DH, :])
            nc.vector.tensor_copy(kbig[DH:2 * DH, :], kbig[0:DH, :])
            vsum_sb = spool.tile([DH, W], BF16, tag="vsum")
            nc.vector.tensor_copy(vsum_sb, ksump[DH:128, :])

            qp_quad(1)

            # ---------- Vbig [128, 12, 68] ----------
            # j-chunk j rows p hold Vbig row 128j+p; col 64 = ones (softmax
            # denominator accumulator); cols 65:68 padding.
            vbig = spool.tile([128, NJ, 68], BF16, tag="vbig")
            nc.vector.memset(vbig[:, :, DH:DH + 1], 1.0)
            for yt in range(4):
                sl = slice(128 * yt, 128 * (yt + 1))
                tps = pspool.tile([128, DH], BF16, tag="ctx", bufs=1,
                                  name=f"tps{yt}")
                nc.tensor.transpose(tps, vsum_sb[:, sl], ident[0:DH, 0:DH])
                nc.vector.tensor_copy(vbig[:, 4 + yt, 0:DH], tps)
                tp7 = pspool.tile([128, DH], BF16, tag="ctx", bufs=1,
                                  name=f"tp7{yt}")
                nc.tensor.transpose(tp7, v7_sb[:, sl], ident[0:DH, 0:DH])
                nc.vector.tensor_sub(vbig[:, 0 + yt, 0:DH],
                                     vbig[:, 4 + yt, 0:DH], tp7)
                tp0 = pspool.tile([128, DH], BF16, tag="ctx", bufs=1,
                                  name=f"tp0{yt}")
                nc.tensor.transpose(tp0, v0_sb[:, sl], ident[0:DH, 0:DH])
                nc.vector.tensor_sub(vbig[:, 8 + yt, 0:DH],
                                     vbig[:, 4 + yt, 0:DH], tp0)

            # ---------- main attention machinery ----------
            outacc = []
            for lt in range(4):
                t = opool.tile([128, DM], BF16, tag="outacc", name=f"outacc{lt}")
                outacc.append(t)
            ctxu_sb = [None] * 8

            pending = []

            def pop_outproj():
                if pending:
                    emit_outproj(*pending.pop(0))

            def emit_outproj(ps_, lt):
                # out-proj of a pair group: the group accumulates into one
                # [128,1024] psum tile on the dedicated "op" bank pair, then
                # a single DVE accumulate into outacc.  Pairs 6 and 7 are
                # their own groups so pair 6's work overlaps pair 7's j-loop
                # and only pair 7's lands in the tail.
                op = pspool.tile([128, 1024], F32, tag="op", bufs=1,
                                 name=f"op{ps_[0]}_{lt}")
                for i, p in enumerate(ps_):
                    cu = ctxu_sb[p]
                    for half in range(2):
                        nc.tensor.matmul(
                            op[:, 512 * half:512 * (half + 1)],
                            cu[:, 128 * lt:128 * (lt + 1)],
                            wc_sb[p][:, 512 * half:512 * (half + 1)],
                            start=(i == 0), stop=(i == len(ps_) - 1))
                if ps_[0] == 0:
                    nc.vector.tensor_copy(outacc[lt], op)
                else:
                    opsb = crpool.tile([128, 1024], BF16, tag="opsb",
                                       bufs=1, name=f"opsb{ps_[0]}_{lt}")
                    nc.vector.tensor_copy(opsb, op)
                    nc.gpsimd.tensor_add(outacc[lt], outacc[lt], opsb)
                if ps_[0] == 7:
                    nc.gpsimd.dma_start(out=out[128 * lt:128 * (lt + 1), :],
                                        in_=outacc[lt])

            def make_ctx(p):
                # merged pair accumulator: cols 0:512 head A, 512:1024 head B
                # (one 1024-wide bf16-rhs matmul per j-chunk feeds both)
                return pspool.tile([128, 1024], F32, tag="ctx", bufs=1,
                                   name=f"ctx{p}")

            def pv_mm(ctx, j, pr, start, stop):
                # two 512-wide matmuls (single-psum-bank rule) sharing lhsT
                nc.tensor.matmul(ctx[0:DH + 1, 0:W], vbig[:, j, 0:DH + 1],
                                 pr[:, 0:W], start=start, stop=stop)
                nc.tensor.matmul(ctx[0:DH + 1, W:2 * W], vbig[:, j, 0:DH + 1],
                                 pr[:, W:2 * W], start=start, stop=stop)

            def finish_pair(p, ctx):
                # raw-evict ctx psum (fast slot release), then normalize
                # lazily from SBUF: row 64 is the softmax denominator.
                cu = cupool.tile([128, W], BF16, tag="ctxu", name=f"ctxu{p}")
                ctxu_sb[p] = cu
                if p in (1, 3, 5):
                    pending.extend(((p - 1, p), lt) for lt in range(4))
                elif p in (6, 7):
                    pending.extend(((p,), lt) for lt in range(4))
                dtmp = mpool.tile([1, 2 * W], F32, tag="dtmp", bufs=1, name=f"dt{p}")
                nc.vector.tensor_copy(dtmp, ctx[DH:DH + 1, :])
                cr = crpool.tile([DH, 2 * W], BF16, tag="craw", name=f"cr{p}")
                nc.vector.tensor_copy(cr, ctx[0:DH, :])
                rc = mpool.tile([1, 2 * W], F32, tag="rc", bufs=1, name=f"rc{p}")
                nc.vector.reciprocal_approx_fast(rc, dtmp)
                bc = mpool.tile([DH, 2 * W], F32, tag="bc", bufs=1, name=f"bc{p}")
                nc.gpsimd.partition_broadcast(bc, rc)
                # normalize on GpSimd (all-SBUF operands) to keep the DVE
                # queue clear at pair boundaries
                nc.gpsimd.tensor_mul(cu[0:DH, :], cr[:, 0:W], bc[:, 0:W])
                nc.gpsimd.tensor_mul(cu[DH:128, :], cr[:, W:2 * W],
                                     bc[:, W:2 * W])

            def beta_scores(p):
                # u-exponentials of a decomposed pair
                qpt = qpt_sb[p // 2]
                csl = slice(512 * (p % 2), 512 * (p % 2) + W)
                eus = []
                for jj in range(4):
                    qk = qk_mm_pair(kbig[:, W + 128 * jj:W + 128 * (jj + 1)],
                                    qpt, csl, f"uqk{p}_{jj}")
                    eu = ppool.tile([128, 1024], BF16, tag="probs",
                                    name=f"eu{p}_{jj}")
                    nc.scalar.activation(eu, qk, AF.Exp, scale=0.125)
                    eus.append(eu)
                return eus

            def beta_combine(p, eus):
                # recombination + PV for a decomposed pair
                ctx = make_ctx(p)
                for jj in range(4):
                    eu = eus[jj]
                    nc.vector.tensor_mul(ea_t[p][jj], ea_t[p][jj], eu)
                    nc.vector.tensor_mul(eb_t[p][jj], eb_t[p][jj], eu)
                    pv_mm(ctx, 4 + jj, eu, start=(jj == 0), stop=False)
                    pv_mm(ctx, 0 + jj, ea_t[p][jj], start=False, stop=False)
                    pv_mm(ctx, 8 + jj, eb_t[p][jj], start=False, stop=(jj == 3))
                finish_pair(p, ctx)

            def std_pair(p, extra=None):
                qpt = qpt_sb[p // 2]
                csl = slice(512 * (p % 2), 512 * (p % 2) + W)
                ctx = make_ctx(p)
                for j in range(NJ):
                    qk = qk_mm_pair(kbig[:, 128 * j:128 * (j + 1)], qpt, csl,
                                    f"qk{p}_{j}")
                    pr = ppool.tile([128, 1024], BF16, tag="probs",
                                    name=f"pr{p}_{j}")
                    nc.scalar.activation(pr, qk, AF.Exp, scale=0.125)
                    pv_mm(ctx, j, pr, start=(j == 0), stop=(j == NJ - 1))
                    if j in (3, 4, 5, 6):
                        pop_outproj()
                    if j == 8 and extra is not None:
                        extra()  # QP quads ride the ACT-saturated j-loop
                finish_pair(p, ctx)

            eus0 = beta_scores(0)
            eus1 = beta_scores(1)
            beta_combine(0, eus0)
            beta_combine(1, eus1)
            std_pair(2, extra=lambda: qp_quad(2))
            std_pair(3, extra=lambda: qp_quad(3))
            for p in range(4, 8):
                std_pair(p)
            while pending:
                pop_outproj()

    nc.compile()
    return nc


_NC = None


def _get_nc():
    global _NC
    if _NC is None:
        _NC = build_nc()
    return _NC


def _pack(a):
    # [1024, cols] -> [128, 8*cols] with packed[p, d*cols+c] = a[128*d+p, c]
    cols = a.shape[1]
    return np.ascontiguousarray(
        a.reshape(DMT, 128, cols).transpose(1, 0, 2).reshape(128, DMT * cols))


def make_in_maps(q, kv, Wq, Wkv, Wc):
    bf = ml_dtypes.bfloat16
    qT_full = np.asarray(q, np.float32)[0].T.astype(bf)
    kvT = np.asarray(kv, np.float32)[0].T.astype(bf)
    Wqb = np.asarray(Wq, np.float32).astype(bf)
    Wkvb = np.asarray(Wkv, np.float32).astype(bf)
    Wcb = np.asarray(Wc, np.float32).astype(bf)
    shared = {
        "kvm": np.ascontiguousarray(kvT[:, W:L - W]),
        "kvc0": _pack(kvT[:, 0:W]),
        "kvc7": _pack(kvT[:, L - W:L]),
        "wqlo": _pack(Wqb[:, 0:512]),
        "wqhi": _pack(Wqb[:, 512:1024]),
        "wkvp": _pack(Wkvb),
        "wcp": _pack(Wcb),
    }
    in_maps = []
    for i in range(N_CORES):
        m = dict(shared)
        m["qtp"] = _pack(qT_full[:, LS * i:LS * (i + 1)])
        in_maps.append(m)
    return in_maps


def kernel(q, kv, Wq, Wkv, Wc, w):
    assert int(w) == W
    q = np.asarray(q, dtype=np.float32)
    B = q.shape[0]
    assert B == 1 and q.shape[1] == L and q.shape[2] == DM

    in_maps = make_in_maps(q, kv, Wq, Wkv, Wc)
    nc = _get_nc()
    res = run_bass_kernel_spmd(nc, in_maps, list(range(N_CORES)))
    out = np.concatenate([np.asarray(res.results[i]["out"], dtype=np.float32)
                          for i in range(N_CORES)], axis=0)
    return out.reshape(1, L, DM)
